# revision 7
# baseline (speedup 1.0000x reference)
"""Block-diagonal complex matmul kernel for trn2 (8 NeuronCores).

Reference computation:
  xp = take(x, perm_idx, axis=-2).reshape(B, 2, M, S)
  y_re = xp_re @ hr1 + xp_im @ hi1   (per block a of M)
  y_im = xp_re @ hi2 + xp_im @ hr2
  out  = stack([y_re, y_im], 1).reshape(B, 2, N, R)

Sharding: block dim M=1024 split across 8 cores (128 blocks each).
Permutation gather + all layout shuffles happen host-side in numpy.

Per-core device kernel, per block a:
  psum[16, 256] = xT_re[:, a] .T @ [hr1[a] | hi2[a]]   (start)
                + xT_im[:, a] .T @ [hi1[a] | hr2[a]]   (stop)
  -> cols 0:128 = y_re[a], cols 128:256 = y_im[a]
"""

import os
import numpy as np

B = 16
N = 4096
R = 32
M = 1024   # blocks
S = 128    # block size (contract dim)
NCORES = 8
MLOC = M // NCORES   # 128 blocks per core
NB = 8               # blocks per weight DMA group (2 MiB per dma_start)
NGRP = MLOC // NB

_NC_CACHE = {}


def _build_nc():
    import concourse.bacc as bacc
    import concourse.bass as bass
    import concourse.mybir as mybir
    from concourse import tile

    mm_dt = mybir.dt.float16
    nc = bacc.Bacc(None, target_bir_lowering=False)

    # x operands: hi/lo fp16 split, layout [S(j), MLOC*B]
    xnames = ["xrh", "xrl", "xih", "xil"]
    xdram = {n: nc.dram_tensor(n, [S, MLOC * B], mm_dt, kind="ExternalInput")
             for n in xnames}
    # weights: per block 1024 fp16 cols = [W1_hi | W2_hi | W1_lo | W2_lo]
    # with W1 = [hr1 | hi2], W2 = [hi1 | hr2]
    WC = 8 * S  # 1024 cols per block
    w = nc.dram_tensor("w", [S, MLOC * WC], mm_dt, kind="ExternalInput")
    y = nc.dram_tensor("y", [B, MLOC * 2 * S], mybir.dt.float32, kind="ExternalOutput")

    with tile.TileContext(nc) as tc:
        with (
            tc.tile_pool(name="xp", bufs=1) as xpool,
            tc.tile_pool(name="wp", bufs=4) as wpool,
            tc.tile_pool(name="op", bufs=3) as opool,
            tc.tile_pool(name="ps", bufs=8, space=bass.MemorySpace.PSUM) as ps,
        ):
            xt = {}
            for n in xnames:
                xt[n] = xpool.tile([S, MLOC * B], mm_dt, name=n, tag=n)
                nc.sync.dma_start(xt[n][:], xdram[n][:])
            for g in range(NGRP):
                wt = wpool.tile([S, NB * WC], mm_dt)
                nc.sync.dma_start(wt[:], w[:, g * NB * WC:(g + 1) * NB * WC])
                ot = opool.tile([B, NB * 2 * S], mybir.dt.float32)
                for i in range(NB):
                    a = g * NB + i
                    c0 = i * WC
                    w1h = wt[:, c0:c0 + 2 * S]
                    w2h = wt[:, c0 + 2 * S:c0 + 4 * S]
                    w1l = wt[:, c0 + 4 * S:c0 + 6 * S]
                    w2l = wt[:, c0 + 6 * S:c0 + 8 * S]
                    xs = slice(a * B, (a + 1) * B)
                    pt = ps.tile([B, 2 * S], mybir.dt.float32)
                    nc.tensor.matmul(pt[:], xt["xrh"][:, xs], w1h, start=True, stop=False)
                    nc.tensor.matmul(pt[:], xt["xih"][:, xs], w2h, start=False, stop=False)
                    nc.tensor.matmul(pt[:], xt["xrl"][:, xs], w1h, start=False, stop=False)
                    nc.tensor.matmul(pt[:], xt["xil"][:, xs], w2h, start=False, stop=False)
                    nc.tensor.matmul(pt[:], xt["xrh"][:, xs], w1l, start=False, stop=False)
                    nc.tensor.matmul(pt[:], xt["xih"][:, xs], w2l, start=False, stop=True)
                    if i % 2 == 0:
                        nc.vector.tensor_copy(ot[:, i * 2 * S:(i + 1) * 2 * S], pt[:])
                    else:
                        nc.scalar.copy(ot[:, i * 2 * S:(i + 1) * 2 * S], pt[:])
                nc.sync.dma_start(y[:, g * NB * 2 * S:(g + 1) * NB * 2 * S], ot[:])
    nc.compile()
    return nc


def kernel(x, hr1, hi1, hr2, hi2, perm_idx):
    from concourse.bass_utils import run_bass_kernel_spmd

    if "nc" not in _NC_CACHE:
        _NC_CACHE["nc"] = _build_nc()
    nc = _NC_CACHE["nc"]

    x = np.asarray(x, dtype=np.float32)
    perm_idx = np.asarray(perm_idx)
    # host-side permutation gather + regroup into M blocks of size S
    xp = x[:, :, perm_idx, :].reshape(B, 2, M, S)

    def split16(v):
        hi = v.astype(np.float16)
        lo = (v - hi.astype(np.float32)).astype(np.float16)
        return hi, lo

    in_maps = []
    for c in range(NCORES):
        a0 = c * MLOC
        sl = slice(a0, a0 + MLOC)
        # [B, MLOC, S] -> [S(j), MLOC, B] -> [S, MLOC*B]
        xre = np.ascontiguousarray(
            np.transpose(xp[:, 0, sl, :], (2, 1, 0))
        ).reshape(S, MLOC * B)
        xim = np.ascontiguousarray(
            np.transpose(xp[:, 1, sl, :], (2, 1, 0))
        ).reshape(S, MLOC * B)
        xrh, xrl = split16(xre)
        xih, xil = split16(xim)
        # W1 = [hr1 | hi2], W2 = [hi1 | hr2]; per block [W1h | W2h | W1l | W2l]
        w1 = np.concatenate([hr1[sl], hi2[sl]], axis=2)
        w2 = np.concatenate([hi1[sl], hr2[sl]], axis=2)
        w1h, w1l = split16(w1)
        w2h, w2l = split16(w2)
        wc = np.concatenate([w1h, w2h, w1l, w2l], axis=2)  # [MLOC, S, 8S]
        wc = np.ascontiguousarray(np.transpose(wc, (1, 0, 2))).reshape(S, MLOC * 8 * S)
        in_maps.append({"xrh": xrh, "xrl": xrl, "xih": xih, "xil": xil, "w": wc})

    trace = bool(os.environ.get("KERNEL_TRACE"))
    kwargs = {}
    if trace:
        kwargs["tmpdir"] = os.environ.get("KERNEL_TRACE_DIR") or None
    res = run_bass_kernel_spmd(nc, in_maps, core_ids=list(range(NCORES)), trace=trace, **kwargs)
    if trace and res.exec_time_ns is not None:
        print(f"HW exec time: {res.exec_time_ns} ns")
        _NC_CACHE["exec_time_ns"] = res.exec_time_ns
        _NC_CACHE["profile"] = res

    out = np.empty((B, 2, M, S), dtype=np.float32)
    for c in range(NCORES):
        a0 = c * MLOC
        yc = res.results[c]["y"].reshape(B, MLOC, 2, S)
        out[:, 0, a0:a0 + MLOC, :] = yc[:, :, 0, :]
        out[:, 1, a0:a0 + MLOC, :] = yc[:, :, 1, :]
    return out.reshape(B, 2, N, R)
